# revision 18
# baseline (speedup 1.0000x reference)
"""Trainium2 Bass kernel for the 2-layer GAT (nn_GAT_47459388621602).

Strategy (8 NeuronCores, SPMD, one NEFF) — v2:
  - Host: add self-loops, assign destination nodes to cores
    (degree-stratified, lo/hi-source-balanced), build per-core padded CSR
    gather index lists (int16, table split in halves for dma_gather).
  - Sharded table build: each core computes h1 = bn(x)@W1 only for its own
    6272-node slab (fp16, 512B rows: [h(128), alpha_src(4), pad]), then an
    AllGather replicates the full 50176-row table to every core. Same for
    layer 2 (rows [msg(160), alpha_src(4), pad]). This removes the
    replicated 392-tile table builds and all per-tile DMAs of v1.
  - Per-destination-block aggregation: dma_gather of 512B source rows,
    exact per-dst segment softmax (per-head max subtraction on device),
    denominator-folded weights, in-place weighted tree-reduction in fp16.
  - Layer-2 slab matmul is fused into the layer-1 aggregation loop (PE is
    idle there), so the h2 AllGather fires as soon as P2 ends.
  - Inputs are shipped sharded + fp16 where possible (~4MB/core vs 33MB).
"""
import os
import time

import numpy as np

import concourse.bacc as bacc
import concourse.mybir as mybir
import concourse.tile as tile
from concourse.bass_utils import run_bass_kernel_spmd
from concourse.library_config import mlp as mlp_library
from concourse.masks import make_identity

N_NODES = 50000
IN_F = 129
HID = 32
HEADS = 4
N_CLS = 40
NEG_SLOPE = 0.2
BN_EPS = 1e-5
NCORES = 8
BLK = 128
NBLK = 49
SLAB = NBLK * BLK           # 6272
NID = NCORES * SLAB         # 50176
HALF = NID // 2             # 25088
MASKVAL = -60000.0          # fp16-representable big-negative logit mask
F2 = 160                    # layer-2 feature width
ROW = 256                   # table row stride in fp16 elements (512 bytes)
GC = 8                      # w-columns (x128 idxs) per dma_gather call (ring limit)

f32 = mybir.dt.float32
f16 = mybir.dt.float16
i16 = mybir.dt.int16

GAT_GC = int(os.environ.get("GAT_GC", str(GC)))
GAT_RING = int(os.environ.get("GAT_RING", "16384"))


# ----------------------------------------------------------------- host prep
def _prep_indices(edge_index):
    src0 = np.asarray(edge_index[0], dtype=np.int64)
    dst0 = np.asarray(edge_index[1], dtype=np.int64)
    loops = np.arange(N_NODES, dtype=np.int64)
    src = np.concatenate([src0, loops])
    dst = np.concatenate([dst0, loops])

    deg = np.bincount(dst, minlength=N_NODES)

    # greedy lo/hi source split balancing each destination's in-edge halves
    out_adj_order = np.argsort(src, kind="stable")
    dst_by_src = dst[out_adj_order]
    s_starts = np.searchsorted(src[out_adj_order], np.arange(N_NODES))
    s_ends = np.searchsorted(src[out_adj_order], np.arange(N_NODES) + 1)
    balance = np.zeros(N_NODES, dtype=np.int64)
    is_lo_node = np.zeros(N_NODES, dtype=bool)
    outdeg = s_ends - s_starts
    cap = N_NODES // 2
    n_lo = n_hi = 0
    for n in np.argsort(-outdeg, kind="stable"):
        nb = dst_by_src[s_starts[n]:s_ends[n]]
        go_lo = balance[nb].sum() <= 0
        if go_lo and n_lo >= cap:
            go_lo = False
        if (not go_lo) and n_hi >= cap:
            go_lo = True
        if go_lo:
            is_lo_node[n] = True
            balance[nb] += 1
            n_lo += 1
        else:
            balance[nb] -= 1
            n_hi += 1

    is_lo_src = is_lo_node[src]
    deglo = np.bincount(dst[is_lo_src], minlength=N_NODES)
    deghi = deg - deglo

    # degree-stratified assignment; residue slot order keeps chunk types
    # aligned across cores so slot-wise max W is tight
    GRP = 4 * BLK
    blocks = {}
    for half in range(2):
        ids = np.where(is_lo_node if half == 0 else ~is_lo_node)[0]
        ids = ids[np.argsort(-deg[ids], kind="stable")]
        n_strata = (len(ids) + GRP - 1) // GRP
        assert n_strata <= NBLK
        core_blocks = [[] for _ in range(4)]
        for s in range(n_strata):
            members = ids[s * GRP: min((s + 1) * GRP, len(ids))]
            m_sorted = members[np.argsort(-deglo[members], kind="stable")]
            chs = np.array_split(m_sorted, 4)
            for t, ch in enumerate(chs):
                core_blocks[(t - s) % 4].append((s, ch))
        for q in range(4):
            core_blocks[q].sort(key=lambda x: (x[0] // 4) * 4 + (x[0] + q) % 4)
            for b in range(NBLK):
                ch = core_blocks[q][b][1] if b < len(core_blocks[q]) else np.array([], dtype=np.int64)
                blk = ch[np.argsort(-deglo[ch], kind="stable")] if len(ch) else ch
                blocks[(half * 4 + q, b)] = blk

    node_cid = np.empty(N_NODES, dtype=np.int64)
    Wlo_qb = np.ones((NCORES, NBLK), dtype=np.int64)
    Whi_qb = np.ones((NCORES, NBLK), dtype=np.int64)
    for q in range(NCORES):
        for b in range(NBLK):
            blk = blocks[(q, b)]
            for jj, n in enumerate(blk):
                node_cid[n] = q * SLAB + b * BLK + jj
            if len(blk):
                Wlo_qb[q, b] = max(1, int(deglo[blk].max()))
                Whi_qb[q, b] = max(1, int(deghi[blk].max()))

    Wlo = Wlo_qb.max(axis=0)
    Whi = Whi_qb.max(axis=0)
    S = int((Wlo + Whi).sum())
    offs = np.zeros(NBLK + 1, dtype=np.int64)
    offs[1:] = np.cumsum(Wlo + Whi)

    # pad slots gather a dedicated mask row (last cid of each table half,
    # guaranteed fake: the last stratum never fills its final slot)
    for q in range(NCORES):
        assert len(blocks[(q, NBLK - 1)]) < BLK, "mask row slot not free"
    idx16 = np.full((NCORES, BLK, S), HALF - 1, dtype=np.int16)

    eorder = np.argsort(node_cid[dst], kind="stable")
    src_cid_sorted = node_cid[src[eorder]]
    dst_cid_sorted = node_cid[dst[eorder]]
    lo_sorted = is_lo_src[eorder]
    starts = np.searchsorted(dst_cid_sorted, np.arange(NID))
    ends = np.searchsorted(dst_cid_sorted, np.arange(NID) + 1)

    for q in range(NCORES):
        qbase = q * SLAB
        for b in range(NBLK):
            o = int(offs[b])
            wl = int(Wlo[b])
            for jj in range(BLK):
                cid = qbase + b * BLK + jj
                e0, e1 = starts[cid], ends[cid]
                ss = src_cid_sorted[e0:e1]
                ll = lo_sorted[e0:e1]
                slo = ss[ll]
                shi = ss[~ll] - HALF
                idx16[q, jj, o:o + len(slo)] = slo.astype(np.int16)
                idx16[q, jj, o + wl:o + wl + len(shi)] = shi.astype(np.int16)

    # compact int16 gather index stream [16, S*8]: per block, lo range then
    # hi range, slot-major wrapped by 16 (device replicates to 128 parts)
    idxc = np.zeros((NCORES, 16, S * 8), dtype=np.int16)
    for q in range(NCORES):
        col = 0
        for b in range(NBLK):
            o = int(offs[b])
            for (w0, w1) in ((0, int(Wlo[b])), (int(Wlo[b]), int(Wlo[b] + Whi[b]))):
                nw = w1 - w0
                sl = idx16[q, :, o + w0:o + w1].T.reshape(nw * BLK)   # slot-major
                idxc[q, :, col:col + nw * 8] = sl.reshape(nw * 8, 16).T
                col += nw * 8
        assert col == S * 8

    return dict(node_cid=node_cid, Wlo=Wlo.astype(int), Whi=Whi.astype(int),
                offs=offs, S=S, idxc=idxc)


# ----------------------------------------------------------------- program
def _build_program(Wlo, Whi, offs, S):
    PHASES = os.environ.get("GAT_PHASES", "1234")
    NB_RUN = int(os.environ.get("GAT_NBLK", str(NBLK)))
    nc = bacc.Bacc("TRN2", target_bir_lowering=False, debug=False,
                   num_devices=NCORES, dynamic_dma_scratch_size=GAT_RING)

    # inputs
    t_xTs = nc.dram_tensor("xTs", [BLK, SLAB], f16, kind="ExternalInput")
    t_xls = nc.dram_tensor("xls", [SLAB, 1], f32, kind="ExternalInput")
    t_W1 = nc.dram_tensor("W1f", [BLK, 136], f16, kind="ExternalInput")
    t_W1r = nc.dram_tensor("W1row", [BLK, 136], f32, kind="ExternalInput")
    t_W2 = nc.dram_tensor("W2f", [BLK, 168], f16, kind="ExternalInput")
    t_csd = nc.dram_tensor("csdb", [BLK, HEADS], f32, kind="ExternalInput")
    t_b1p = nc.dram_tensor("b1pb", [BLK, BLK], f16, kind="ExternalInput")
    t_idxc = nc.dram_tensor("idxc", [16, S * 8], i16, kind="ExternalInput")
    t_c2b = nc.dram_tensor("c2b", [BLK, 168], f32, kind="ExternalInput")
    t_out = nc.dram_tensor("out2", [SLAB, F2], f16, kind="ExternalOutput")

    with tile.TileContext(nc) as tc:
        with (
            tc.tile_pool(name="const", bufs=1) as cpool,
            tc.tile_pool(name="dram", bufs=1, space="DRAM") as dpool,
        ):
            nc.gpsimd.load_library(mlp_library)

            # internal DRAM
            h1slab = dpool.tile([SLAB, ROW], f16)
            h1tab = dpool.tile([NID, ROW], f16, addr_space="Shared")
            h2slab = dpool.tile([SLAB, ROW], f16)
            h2tab = dpool.tile([NID, ROW], f16, addr_space="Shared")

            # resident constants / state
            W1sb = cpool.tile([BLK, 136], f16)
            nc.sync.dma_start(out=W1sb[:], in_=t_W1[:])
            W1rsb = cpool.tile([BLK, 136], f32)
            nc.sync.dma_start(out=W1rsb[:], in_=t_W1r[:])
            W2sb = cpool.tile([BLK, 168], f16)
            nc.sync.dma_start(out=W2sb[:], in_=t_W2[:])
            csdsb = cpool.tile([BLK, HEADS], f32)
            nc.sync.dma_start(out=csdsb[:], in_=t_csd[:])
            b1psb = cpool.tile([BLK, BLK], f16)
            nc.sync.dma_start(out=b1psb[:], in_=t_b1p[:])
            c2sb = cpool.tile([BLK, 168], f32)
            nc.sync.dma_start(out=c2sb[:], in_=t_c2b[:])
            mrow = cpool.tile([BLK, HEADS], f16)
            nc.vector.memset(mrow[:], MASKVAL)
            idxw_sb = cpool.tile([BLK, S * 8], i16)
            for g in range(8):
                nc.sync.dma_start(out=idxw_sb[:][16 * g:16 * (g + 1), :],
                                  in_=t_idxc[:])
            ident = cpool.tile([BLK, BLK], f16)
            make_identity(nc, ident[:])
            xTslab = cpool.tile([BLK, SLAB], f16)
            nc.sync.dma_start(out=xTslab[:], in_=t_xTs[:])
            xlall = cpool.tile([BLK, NBLK], f32)
            nc.sync.dma_start(
                out=xlall[:],
                in_=t_xls[:].rearrange("(t p) o -> p (t o)", p=BLK))
            ldc1 = cpool.tile([BLK, NBLK * HEADS], f32)
            ld2 = cpool.tile([BLK, NBLK * HEADS], f32)
            x2T = cpool.tile([BLK, SLAB], f16)

            # ---------------- P1: own-slab h1 (+alpha_src,+alpha_dst)
            with (
                tc.tile_pool(name="p1", bufs=3) as pool,
                tc.tile_pool(name="p1st", bufs=1) as stpool,
                tc.tile_pool(name="p1ps", bufs=2, space="PSUM") as pspool,
            ):
                h1st = stpool.tile([BLK, NBLK * 132], f16)
                for t in range(NBLK if "1" in PHASES else 0):
                    ps = pspool.tile([BLK, 136], f32)
                    nc.tensor.matmul(out=ps[:],
                                     lhsT=xTslab[:, t * BLK:(t + 1) * BLK],
                                     rhs=W1sb[:], start=True, stop=True)
                    r1 = pool.tile([BLK, 136], f32, tag="r1")
                    nc.vector.tensor_scalar_mul(out=r1[:], in0=W1rsb[:],
                                                scalar1=xlall[:, t:t + 1])
                    hs = pool.tile([BLK, 136], f32, tag="hs")
                    nc.vector.tensor_tensor(out=hs[:], in0=ps[:], in1=r1[:],
                                            op=mybir.AluOpType.add)
                    nc.vector.tensor_copy(out=h1st[:, 132 * t:132 * (t + 1)],
                                          in_=hs[:, 0:132])
                    nc.vector.tensor_tensor(
                        out=ldc1[:, HEADS * t:HEADS * (t + 1)],
                        in0=hs[:, 132:136], in1=csdsb[:],
                        op=mybir.AluOpType.add)

                if "1" in PHASES:
                    nc.sync.dma_start(
                        out=h1slab[:, 0:132].rearrange("(t p) f -> p t f", p=BLK),
                        in_=h1st[:].rearrange("p (t f) -> p t f", f=132))
                    nc.sync.dma_start(out=h1slab[SLAB - 1:SLAB, BLK:BLK + HEADS],
                                      in_=mrow[:][0:1, :])
            if "3" in PHASES:
                nc.gpsimd.collective_compute(
                    "AllGather", mybir.AluOpType.bypass,
                    replica_groups=[list(range(NCORES))],
                    ins=[h1slab.opt()], outs=[h1tab.opt()])

            # ---------------- P2: layer-1 aggregation + fused layer-2 matmul
            with (
                tc.tile_pool(name="p2g", bufs=6) as gpool,
                tc.tile_pool(name="p2s", bufs=3) as spool,
                tc.tile_pool(name="p2ps", bufs=4, space="PSUM") as pspool,
                tc.tile_pool(name="p2ps2", bufs=4, space="PSUM") as pspool2,
            ):
                for b in range(NB_RUN if "2" in PHASES else 0):
                    wl, wh = int(Wlo[b]), int(Whi[b])
                    wt = wl + wh
                    o = int(offs[b])
                    G = gpool.tile([BLK, wt * ROW], f16, tag="G")
                    G3 = G[:].rearrange("p (w f) -> p w f", f=ROW)
                    for (wbase, wlen, tab) in [(0, wl, h1tab[0:HALF, :]),
                                               (wl, wh, h1tab[HALF:NID, :])]:
                        for w0 in range(0, wlen, GAT_GC):
                            wn = min(GAT_GC, wlen - w0)
                            nc.gpsimd.dma_gather(
                                G3[:, wbase + w0:wbase + w0 + wn, :], tab,
                                idxw_sb[:, (o + wbase + w0) * 8:(o + wbase + w0 + wn) * 8],
                                wn * BLK, wn * BLK, ROW)
                    # logits: leaky(alpha_src + alpha_dst), head-major
                    lst = spool.tile([BLK, wt * HEADS], f32, tag="lst")
                    lsthw = lst[:].rearrange("p (h w) -> p h w", h=HEADS)
                    nc.vector.tensor_tensor(
                        out=lsthw,
                        in0=G3[:, :, BLK:BLK + HEADS].rearrange("p w h -> p h w"),
                        in1=ldc1[:, HEADS * b:HEADS * (b + 1)].unsqueeze(2)
                            .to_broadcast([BLK, HEADS, wt]),
                        op=mybir.AluOpType.add)
                    tmp = spool.tile([BLK, wt * HEADS], f32, tag="tmp")
                    nc.vector.tensor_scalar_mul(out=tmp[:], in0=lst[:],
                                                scalar1=NEG_SLOPE)
                    nc.vector.tensor_tensor(out=lst[:], in0=lst[:], in1=tmp[:],
                                            op=mybir.AluOpType.max)
                    # exact per-dst segment softmax: subtract per-head max
                    nmx = spool.tile([BLK, HEADS], f32, tag="nmx")
                    nc.vector.tensor_reduce(out=nmx[:], in_=lsthw,
                                            axis=mybir.AxisListType.X,
                                            op=mybir.AluOpType.max)
                    nc.vector.tensor_scalar_mul(out=nmx[:], in0=nmx[:],
                                                scalar1=-1.0)
                    # exp, broadcast-expanded to [p, w, h, c] on ACT
                    pexp = spool.tile([BLK, wt * BLK], f16, tag="pexp")
                    pexp4 = pexp[:].rearrange("p (w h c) -> p w h c",
                                              h=HEADS, c=HID)
                    den = spool.tile([BLK, HEADS], f32, tag="den")
                    for h in range(HEADS):
                        nc.scalar.activation(
                            out=pexp4[:, :, h, :],
                            in_=lst[:, h * wt:(h + 1) * wt].unsqueeze(2)
                                .to_broadcast([BLK, wt, HID]),
                            func=mybir.ActivationFunctionType.Exp,
                            bias=nmx[:, h:h + 1],
                            accum_out=den[:, h:h + 1])
                    nc.vector.tensor_scalar_mul(out=den[:], in0=den[:],
                                                scalar1=1.0 / HID)
                    rcp = spool.tile([BLK, HEADS], f32, tag="rcp")
                    nc.vector.reciprocal(out=rcp[:], in_=den[:])
                    # weighted messages in place (both fp16-packed: 2x DVE)
                    nc.vector.tensor_tensor(
                        out=G3[:, :, 0:BLK], in0=G3[:, :, 0:BLK],
                        in1=pexp[:].rearrange("p (w f) -> p w f", f=BLK),
                        op=mybir.AluOpType.mult)
                    w = wt
                    while w > 1:
                        hsz = w // 2
                        nc.vector.tensor_tensor(
                            out=G3[:, 0:hsz, 0:BLK], in0=G3[:, 0:hsz, 0:BLK],
                            in1=G3[:, w - hsz:w, 0:BLK],
                            op=mybir.AluOpType.add)
                        w -= hsz
                    # normalize + bias
                    x2 = spool.tile([BLK, BLK], f16, tag="x2")
                    nc.vector.tensor_tensor(
                        out=x2[:].rearrange("p (h c) -> p h c", c=HID),
                        in0=G3[:, 0, 0:BLK].rearrange("p (h c) -> p h c", c=HID),
                        in1=rcp[:].unsqueeze(2).to_broadcast([BLK, HEADS, HID]),
                        op=mybir.AluOpType.mult)
                    nc.vector.tensor_tensor(out=x2[:], in0=x2[:], in1=b1psb[:],
                                            op=mybir.AluOpType.add)
                    # elu + 1 (the -1 is folded into the layer-2 correction)
                    ex = spool.tile([BLK, BLK], f16, tag="ex")
                    nc.vector.tensor_scalar_min(out=ex[:], in0=x2[:], scalar1=0.0)
                    exf = spool.tile([BLK, BLK], f16, tag="exf")
                    nc.scalar.activation(out=exf[:], in_=ex[:],
                                         func=mybir.ActivationFunctionType.Exp)
                    nc.vector.tensor_scalar_max(out=x2[:], in0=x2[:], scalar1=0.0)
                    nc.vector.tensor_tensor(out=x2[:], in0=x2[:], in1=exf[:],
                                            op=mybir.AluOpType.add)
                    # transpose -> resident x2T slab (fp16)
                    tps = pspool.tile([BLK, BLK], f16, tag="tps")
                    nc.tensor.transpose(out=tps[:], in_=x2[:], identity=ident[:])
                    nc.scalar.activation(out=x2T[:, b * BLK:(b + 1) * BLK],
                                         in_=tps[:],
                                         func=mybir.ActivationFunctionType.Copy)
                    # fused layer-2 slab matmul (+ elu-shift correction)
                    ps2 = pspool2.tile([BLK, 168], f32)
                    nc.tensor.matmul(out=ps2[:],
                                     lhsT=x2T[:, b * BLK:(b + 1) * BLK],
                                     rhs=W2sb[:], start=True, stop=True)
                    h2c = spool.tile([BLK, 168], f16, tag="h2c")
                    nc.vector.tensor_tensor(out=h2c[:], in0=ps2[:], in1=c2sb[:],
                                            op=mybir.AluOpType.add)
                    nc.sync.dma_start(out=h2slab[b * BLK:(b + 1) * BLK, 0:168],
                                      in_=h2c[:])
                    nc.vector.tensor_tensor(out=ld2[:, HEADS * b:HEADS * (b + 1)],
                                            in0=ps2[:, 164:168],
                                            in1=c2sb[:, 164:168],
                                            op=mybir.AluOpType.add)

            if "2" in PHASES:
                nc.sync.dma_start(out=h2slab[SLAB - 1:SLAB, F2:F2 + HEADS],
                                  in_=mrow[:][0:1, :])
            if "3" in PHASES:
                nc.gpsimd.collective_compute(
                    "AllGather", mybir.AluOpType.bypass,
                    replica_groups=[list(range(NCORES))],
                    ins=[h2slab.opt()], outs=[h2tab.opt()])

            # ---------------- P4: layer-2 aggregation -> out
            with (
                tc.tile_pool(name="p4g", bufs=6) as gpool,
                tc.tile_pool(name="p4s", bufs=3) as spool,
            ):
                for b in range(NB_RUN if "4" in PHASES else 0):
                    wl, wh = int(Wlo[b]), int(Whi[b])
                    wt = wl + wh
                    o = int(offs[b])
                    G = gpool.tile([BLK, wt * ROW], f16, tag="G2")
                    G3 = G[:].rearrange("p (w f) -> p w f", f=ROW)
                    for (wbase, wlen, tab) in [(0, wl, h2tab[0:HALF, :]),
                                               (wl, wh, h2tab[HALF:NID, :])]:
                        for w0 in range(0, wlen, GAT_GC):
                            wn = min(GAT_GC, wlen - w0)
                            nc.gpsimd.dma_gather(
                                G3[:, wbase + w0:wbase + w0 + wn, :], tab,
                                idxw_sb[:, (o + wbase + w0) * 8:(o + wbase + w0 + wn) * 8],
                                wn * BLK, wn * BLK, ROW)
                    lst = spool.tile([BLK, wt * HEADS], f32, tag="lst2")
                    lsthw = lst[:].rearrange("p (h w) -> p h w", h=HEADS)
                    nc.vector.tensor_tensor(
                        out=lsthw,
                        in0=G3[:, :, F2:F2 + HEADS].rearrange("p w h -> p h w"),
                        in1=ld2[:, HEADS * b:HEADS * (b + 1)].unsqueeze(2)
                            .to_broadcast([BLK, HEADS, wt]),
                        op=mybir.AluOpType.add)
                    tmp = spool.tile([BLK, wt * HEADS], f32, tag="tmp2")
                    nc.vector.tensor_scalar_mul(out=tmp[:], in0=lst[:],
                                                scalar1=NEG_SLOPE)
                    nc.vector.tensor_tensor(out=lst[:], in0=lst[:], in1=tmp[:],
                                            op=mybir.AluOpType.max)
                    nmx = spool.tile([BLK, HEADS], f32, tag="nmx2")
                    nc.vector.tensor_reduce(out=nmx[:], in_=lsthw,
                                            axis=mybir.AxisListType.X,
                                            op=mybir.AluOpType.max)
                    nc.vector.tensor_scalar_mul(out=nmx[:], in0=nmx[:],
                                                scalar1=-1.0)
                    pexp = spool.tile([BLK, wt * F2], f16, tag="pexp2")
                    pexp4 = pexp[:].rearrange("p (w h c) -> p w h c",
                                              h=HEADS, c=N_CLS)
                    den = spool.tile([BLK, HEADS], f32, tag="den2")
                    for h in range(HEADS):
                        nc.scalar.activation(
                            out=pexp4[:, :, h, :],
                            in_=lst[:, h * wt:(h + 1) * wt].unsqueeze(2)
                                .to_broadcast([BLK, wt, N_CLS]),
                            func=mybir.ActivationFunctionType.Exp,
                            bias=nmx[:, h:h + 1],
                            accum_out=den[:, h:h + 1])
                    nc.vector.tensor_scalar_mul(out=den[:], in0=den[:],
                                                scalar1=1.0 / N_CLS)
                    rcp = spool.tile([BLK, HEADS], f32, tag="rcp2")
                    nc.vector.reciprocal(out=rcp[:], in_=den[:])
                    nc.vector.tensor_tensor(
                        out=G3[:, :, 0:F2], in0=G3[:, :, 0:F2],
                        in1=pexp[:].rearrange("p (w f) -> p w f", f=F2),
                        op=mybir.AluOpType.mult)
                    w = wt
                    while w > 1:
                        hsz = w // 2
                        nc.vector.tensor_tensor(
                            out=G3[:, 0:hsz, 0:F2], in0=G3[:, 0:hsz, 0:F2],
                            in1=G3[:, w - hsz:w, 0:F2],
                            op=mybir.AluOpType.add)
                        w -= hsz
                    ot = spool.tile([BLK, F2], f16, tag="ot")
                    nc.vector.tensor_tensor(
                        out=ot[:].rearrange("p (h c) -> p h c", c=N_CLS),
                        in0=G3[:, 0, 0:F2].rearrange("p (h c) -> p h c", c=N_CLS),
                        in1=rcp[:].unsqueeze(2).to_broadcast([BLK, HEADS, N_CLS]),
                        op=mybir.AluOpType.mult)
                    nc.sync.dma_start(out=t_out[b * BLK:(b + 1) * BLK, :],
                                      in_=ot[:])

    nc.compile()
    return nc


_CACHE = {}


def kernel(**inputs) -> np.ndarray:
    x = np.asarray(inputs["x"], np.float32)
    P = _prep_indices(np.asarray(inputs["edge_index"]))
    node_cid = P["node_cid"]

    g = np.asarray(inputs["bn_gamma"], np.float32)
    bta = np.asarray(inputs["bn_beta"], np.float32)
    mu = np.asarray(inputs["bn_mean"], np.float32)
    var = np.asarray(inputs["bn_var"], np.float32)
    W1 = np.asarray(inputs["W1"], np.float32)
    a1s = np.asarray(inputs["a1_src"], np.float32)
    a1d = np.asarray(inputs["a1_dst"], np.float32)
    W2 = np.asarray(inputs["W2"], np.float32)
    a2s = np.asarray(inputs["a2_src"], np.float32)
    a2d = np.asarray(inputs["a2_dst"], np.float32)

    s = g / np.sqrt(var + BN_EPS)
    W1p = (s[:, None] * W1).astype(np.float32)
    b1p = ((bta - mu * s) @ W1).astype(np.float32)
    A1s = np.zeros((HEADS * HID, HEADS), np.float32)
    A1d = np.zeros((HEADS * HID, HEADS), np.float32)
    A2s = np.zeros((HEADS * N_CLS, HEADS), np.float32)
    A2d = np.zeros((HEADS * N_CLS, HEADS), np.float32)
    for h in range(HEADS):
        A1s[h * HID:(h + 1) * HID, h] = a1s[h]
        A1d[h * HID:(h + 1) * HID, h] = a1d[h]
        A2s[h * N_CLS:(h + 1) * N_CLS, h] = a2s[h]
        A2d[h * N_CLS:(h + 1) * N_CLS, h] = a2d[h]
    W1f = np.concatenate([W1p, W1p @ A1s, W1p @ A1d], axis=1)      # [129, 136]
    csd = (b1p @ A1s + b1p @ A1d).astype(np.float32)               # [4]
    W2f = np.concatenate([W2, W2 @ A2s, W2 @ A2d], axis=1)         # [128, 168]

    # x in cid space
    xp = np.zeros((NID, IN_F), np.float32)
    xp[node_cid] = x

    key = (tuple(P["Wlo"]), tuple(P["Whi"]),
           os.environ.get("GAT_PHASES", "1234"), os.environ.get("GAT_NBLK", ""))
    if key not in _CACHE:
        _CACHE[key] = _build_program(P["Wlo"], P["Whi"], P["offs"], P["S"])
    nc = _CACHE[key]

    common = {
        "W1f": np.ascontiguousarray(W1f[:BLK]).astype(np.float16),
        "W1row": np.tile(W1f[BLK:BLK + 1], (BLK, 1)).astype(np.float32),
        "W2f": W2f.astype(np.float16),
        "csdb": np.tile(csd[None, :], (BLK, 1)).astype(np.float32),
        "b1pb": np.tile(b1p[None, :], (BLK, 1)).astype(np.float16),
        "c2b": np.tile(-W2f.sum(axis=0)[None, :], (BLK, 1)).astype(np.float32),
    }

    in_maps = []
    for q in range(NCORES):
        m = dict(common)
        sl = slice(q * SLAB, (q + 1) * SLAB)
        m["xTs"] = np.ascontiguousarray(xp[sl, 0:BLK].T).astype(np.float16)
        m["xls"] = np.ascontiguousarray(xp[sl, BLK:BLK + 1]).astype(np.float32)
        m["idxc"] = np.ascontiguousarray(P["idxc"][q])
        in_maps.append(m)

    t0 = time.time()
    res = run_bass_kernel_spmd(nc, in_maps, core_ids=list(range(NCORES)))
    global last_run_seconds
    last_run_seconds = time.time() - t0
    outfull = np.concatenate([r["out2"] for r in res.results], axis=0)
    return outfull[node_cid].astype(np.float32)


last_run_seconds = None


# revision 19
# speedup vs baseline: 2.0333x; 2.0333x over previous
"""Trainium2 Bass kernel for the 2-layer GAT (nn_GAT_47459388621602).

Strategy (8 NeuronCores, SPMD, one NEFF) — v3:
  - Host: add self-loops, assign destination nodes to cores
    (degree-stratified, lo/hi-source-balanced), build per-core padded CSR
    gather index lists (int16, table split in halves for dma_gather).
    Padded slots point at a dedicated mask row (alpha_src = -60000) so no
    separate mask tensor or mask-add op is needed.
  - Sharded table build: each core computes h1 = bn(x)@W1 only for its own
    6272-node slab (fp16, 512B rows: [h(128), alpha_src(4), pad]), then an
    AllGather replicates the full 50176-row table to every core. Same for
    layer 2 (rows [msg(160), alpha_src(4), pad]).
  - Per-destination-block aggregation: dma_gather of 512B source rows;
    exact per-dst segment softmax (per-head max subtraction); the exp is
    broadcast-expanded on the ACT engine into a [dst, slot, head, ch] fp16
    weight tile so the weighted-message multiply runs in the DVE 2x fp16
    mode; in-place fp16 tree-reduction; normalization folded after the
    reduce. ELU is 3 DVE ops + 1 ACT exp with its -1 folded into a
    layer-2 column correction (c2b).
  - Layer-2 slab matmul is fused into the layer-1 aggregation loop (PE is
    idle there), so the h2 AllGather fires as soon as P2 ends.
  - Inputs are shipped sharded + fp16 where possible (~4MB/core vs 33MB).
  - dma_gather ring stays at the default 16384 bytes / 1024 idxs per call
    (GC=8): larger rings hang the device.
"""
import os
import time

import numpy as np

import concourse.bacc as bacc
import concourse.mybir as mybir
import concourse.tile as tile
from concourse.bass_utils import run_bass_kernel_spmd
from concourse.library_config import mlp as mlp_library
from concourse.masks import make_identity

N_NODES = 50000
IN_F = 129
HID = 32
HEADS = 4
N_CLS = 40
NEG_SLOPE = 0.2
BN_EPS = 1e-5
NCORES = 8
BLK = 128
NBLK = 49
SLAB = NBLK * BLK           # 6272
NID = NCORES * SLAB         # 50176
HALF = NID // 2             # 25088
MASKVAL = -60000.0          # fp16-representable big-negative logit mask
F2 = 160                    # layer-2 feature width
ROW = 256                   # table row stride in fp16 elements (512 bytes)
GC = 8                      # w-columns (x128 idxs) per dma_gather call (ring limit)

f32 = mybir.dt.float32
f16 = mybir.dt.float16
i16 = mybir.dt.int16

GAT_GC = int(os.environ.get("GAT_GC", str(GC)))
GAT_RING = int(os.environ.get("GAT_RING", "16384"))


# ----------------------------------------------------------------- host prep
def _prep_indices(edge_index):
    src0 = np.asarray(edge_index[0], dtype=np.int64)
    dst0 = np.asarray(edge_index[1], dtype=np.int64)
    loops = np.arange(N_NODES, dtype=np.int64)
    src = np.concatenate([src0, loops])
    dst = np.concatenate([dst0, loops])

    deg = np.bincount(dst, minlength=N_NODES)

    # greedy lo/hi source split balancing each destination's in-edge halves
    out_adj_order = np.argsort(src, kind="stable")
    dst_by_src = dst[out_adj_order]
    s_starts = np.searchsorted(src[out_adj_order], np.arange(N_NODES))
    s_ends = np.searchsorted(src[out_adj_order], np.arange(N_NODES) + 1)
    balance = np.zeros(N_NODES, dtype=np.int64)
    is_lo_node = np.zeros(N_NODES, dtype=bool)
    outdeg = s_ends - s_starts
    cap = N_NODES // 2
    n_lo = n_hi = 0
    for n in np.argsort(-outdeg, kind="stable"):
        nb = dst_by_src[s_starts[n]:s_ends[n]]
        go_lo = balance[nb].sum() <= 0
        if go_lo and n_lo >= cap:
            go_lo = False
        if (not go_lo) and n_hi >= cap:
            go_lo = True
        if go_lo:
            is_lo_node[n] = True
            balance[nb] += 1
            n_lo += 1
        else:
            balance[nb] -= 1
            n_hi += 1

    is_lo_src = is_lo_node[src]
    deglo = np.bincount(dst[is_lo_src], minlength=N_NODES)
    deghi = deg - deglo

    # degree-stratified assignment; residue slot order keeps chunk types
    # aligned across cores so slot-wise max W is tight
    GRP = 4 * BLK
    blocks = {}
    for half in range(2):
        ids = np.where(is_lo_node if half == 0 else ~is_lo_node)[0]
        ids = ids[np.argsort(-deg[ids], kind="stable")]
        n_strata = (len(ids) + GRP - 1) // GRP
        assert n_strata <= NBLK
        core_blocks = [[] for _ in range(4)]
        for s in range(n_strata):
            members = ids[s * GRP: min((s + 1) * GRP, len(ids))]
            m_sorted = members[np.argsort(-deglo[members], kind="stable")]
            chs = np.array_split(m_sorted, 4)
            for t, ch in enumerate(chs):
                core_blocks[(t - s) % 4].append((s, ch))
        for q in range(4):
            core_blocks[q].sort(key=lambda x: (x[0] // 4) * 4 + (x[0] + q) % 4)
            for b in range(NBLK):
                ch = core_blocks[q][b][1] if b < len(core_blocks[q]) else np.array([], dtype=np.int64)
                blk = ch[np.argsort(-deglo[ch], kind="stable")] if len(ch) else ch
                blocks[(half * 4 + q, b)] = blk

    node_cid = np.empty(N_NODES, dtype=np.int64)
    Wlo_qb = np.ones((NCORES, NBLK), dtype=np.int64)
    Whi_qb = np.ones((NCORES, NBLK), dtype=np.int64)
    for q in range(NCORES):
        for b in range(NBLK):
            blk = blocks[(q, b)]
            for jj, n in enumerate(blk):
                node_cid[n] = q * SLAB + b * BLK + jj
            if len(blk):
                Wlo_qb[q, b] = max(1, int(deglo[blk].max()))
                Whi_qb[q, b] = max(1, int(deghi[blk].max()))

    Wlo = Wlo_qb.max(axis=0)
    Whi = Whi_qb.max(axis=0)
    S = int((Wlo + Whi).sum())
    offs = np.zeros(NBLK + 1, dtype=np.int64)
    offs[1:] = np.cumsum(Wlo + Whi)

    # pad slots gather a dedicated mask row (last cid of each table half,
    # guaranteed fake: the last stratum never fills its final slot)
    for q in range(NCORES):
        assert len(blocks[(q, NBLK - 1)]) < BLK, "mask row slot not free"
    idx16 = np.full((NCORES, BLK, S), HALF - 1, dtype=np.int16)

    eorder = np.argsort(node_cid[dst], kind="stable")
    src_cid_sorted = node_cid[src[eorder]]
    dst_cid_sorted = node_cid[dst[eorder]]
    lo_sorted = is_lo_src[eorder]
    starts = np.searchsorted(dst_cid_sorted, np.arange(NID))
    ends = np.searchsorted(dst_cid_sorted, np.arange(NID) + 1)

    for q in range(NCORES):
        qbase = q * SLAB
        for b in range(NBLK):
            o = int(offs[b])
            wl = int(Wlo[b])
            for jj in range(BLK):
                cid = qbase + b * BLK + jj
                e0, e1 = starts[cid], ends[cid]
                ss = src_cid_sorted[e0:e1]
                ll = lo_sorted[e0:e1]
                slo = ss[ll]
                shi = ss[~ll] - HALF
                idx16[q, jj, o:o + len(slo)] = slo.astype(np.int16)
                idx16[q, jj, o + wl:o + wl + len(shi)] = shi.astype(np.int16)

    # compact int16 gather index stream [16, S*8]: per block, lo range then
    # hi range, slot-major wrapped by 16 (device replicates to 128 parts)
    idxc = np.zeros((NCORES, 16, S * 8), dtype=np.int16)
    for q in range(NCORES):
        col = 0
        for b in range(NBLK):
            o = int(offs[b])
            for (w0, w1) in ((0, int(Wlo[b])), (int(Wlo[b]), int(Wlo[b] + Whi[b]))):
                nw = w1 - w0
                sl = idx16[q, :, o + w0:o + w1].T.reshape(nw * BLK)   # slot-major
                idxc[q, :, col:col + nw * 8] = sl.reshape(nw * 8, 16).T
                col += nw * 8
        assert col == S * 8

    return dict(node_cid=node_cid, Wlo=Wlo.astype(int), Whi=Whi.astype(int),
                offs=offs, S=S, idxc=idxc)


# ----------------------------------------------------------------- program
def _build_program(Wlo, Whi, offs, S):
    PHASES = os.environ.get("GAT_PHASES", "1234")
    NB_RUN = int(os.environ.get("GAT_NBLK", str(NBLK)))
    nc = bacc.Bacc("TRN2", target_bir_lowering=False, debug=False,
                   num_devices=NCORES, dynamic_dma_scratch_size=GAT_RING)

    # inputs
    t_xTs = nc.dram_tensor("xTs", [BLK, SLAB], f16, kind="ExternalInput")
    t_xls = nc.dram_tensor("xls", [SLAB, 1], f32, kind="ExternalInput")
    t_W1 = nc.dram_tensor("W1f", [BLK, 136], f16, kind="ExternalInput")
    t_W1r = nc.dram_tensor("W1row", [BLK, 136], f32, kind="ExternalInput")
    t_W2 = nc.dram_tensor("W2f", [BLK, 168], f16, kind="ExternalInput")
    t_csd = nc.dram_tensor("csdb", [BLK, HEADS], f32, kind="ExternalInput")
    t_b1p = nc.dram_tensor("b1pb", [BLK, BLK], f16, kind="ExternalInput")
    t_idxc = nc.dram_tensor("idxc", [16, S * 8], i16, kind="ExternalInput")
    t_c2b = nc.dram_tensor("c2b", [BLK, 168], f32, kind="ExternalInput")
    t_out = nc.dram_tensor("out2", [SLAB, F2], f16, kind="ExternalOutput")

    with tile.TileContext(nc) as tc:
        with (
            tc.tile_pool(name="const", bufs=1) as cpool,
            tc.tile_pool(name="dram", bufs=1, space="DRAM") as dpool,
        ):
            nc.gpsimd.load_library(mlp_library)

            # internal DRAM
            h1slab = dpool.tile([SLAB, ROW], f16)
            h1tab = dpool.tile([NID, ROW], f16, addr_space="Shared")
            h2slab = dpool.tile([SLAB, ROW], f16)
            h2tab = dpool.tile([NID, ROW], f16, addr_space="Shared")

            # resident constants / state
            W1sb = cpool.tile([BLK, 136], f16)
            nc.sync.dma_start(out=W1sb[:], in_=t_W1[:])
            W1rsb = cpool.tile([BLK, 136], f32)
            nc.sync.dma_start(out=W1rsb[:], in_=t_W1r[:])
            W2sb = cpool.tile([BLK, 168], f16)
            nc.sync.dma_start(out=W2sb[:], in_=t_W2[:])
            csdsb = cpool.tile([BLK, HEADS], f32)
            nc.sync.dma_start(out=csdsb[:], in_=t_csd[:])
            b1psb = cpool.tile([BLK, BLK], f16)
            nc.sync.dma_start(out=b1psb[:], in_=t_b1p[:])
            c2sb = cpool.tile([BLK, 168], f32)
            nc.sync.dma_start(out=c2sb[:], in_=t_c2b[:])
            mrow = cpool.tile([BLK, HEADS], f16)
            nc.vector.memset(mrow[:], MASKVAL)
            idxw_sb = cpool.tile([BLK, S * 8], i16)
            for g in range(8):
                nc.sync.dma_start(out=idxw_sb[:][16 * g:16 * (g + 1), :],
                                  in_=t_idxc[:])
            ident = cpool.tile([BLK, BLK], f16)
            make_identity(nc, ident[:])
            xTslab = cpool.tile([BLK, SLAB], f16)
            nc.sync.dma_start(out=xTslab[:], in_=t_xTs[:])
            xlall = cpool.tile([BLK, NBLK], f32)
            nc.sync.dma_start(
                out=xlall[:],
                in_=t_xls[:].rearrange("(t p) o -> p (t o)", p=BLK))
            ldc1 = cpool.tile([BLK, NBLK * HEADS], f32)
            ld2 = cpool.tile([BLK, NBLK * HEADS], f32)
            x2T = cpool.tile([BLK, SLAB], f16)

            # ---------------- P1: own-slab h1 (+alpha_src,+alpha_dst)
            with (
                tc.tile_pool(name="p1", bufs=3) as pool,
                tc.tile_pool(name="p1st", bufs=1) as stpool,
                tc.tile_pool(name="p1ps", bufs=2, space="PSUM") as pspool,
            ):
                h1st = stpool.tile([BLK, NBLK * 132], f16)
                for t in range(NBLK if "1" in PHASES else 0):
                    ps = pspool.tile([BLK, 136], f32)
                    nc.tensor.matmul(out=ps[:],
                                     lhsT=xTslab[:, t * BLK:(t + 1) * BLK],
                                     rhs=W1sb[:], start=True, stop=True)
                    r1 = pool.tile([BLK, 136], f32, tag="r1")
                    nc.vector.tensor_scalar_mul(out=r1[:], in0=W1rsb[:],
                                                scalar1=xlall[:, t:t + 1])
                    hs = pool.tile([BLK, 136], f32, tag="hs")
                    nc.vector.tensor_tensor(out=hs[:], in0=ps[:], in1=r1[:],
                                            op=mybir.AluOpType.add)
                    nc.vector.tensor_copy(out=h1st[:, 132 * t:132 * (t + 1)],
                                          in_=hs[:, 0:132])
                    nc.vector.tensor_tensor(
                        out=ldc1[:, HEADS * t:HEADS * (t + 1)],
                        in0=hs[:, 132:136], in1=csdsb[:],
                        op=mybir.AluOpType.add)

                if "1" in PHASES:
                    nc.sync.dma_start(
                        out=h1slab[:, 0:132].rearrange("(t p) f -> p t f", p=BLK),
                        in_=h1st[:].rearrange("p (t f) -> p t f", f=132))
                    nc.sync.dma_start(out=h1slab[SLAB - 1:SLAB, BLK:BLK + HEADS],
                                      in_=mrow[:][0:1, :])
            if "3" in PHASES:
                nc.gpsimd.collective_compute(
                    "AllGather", mybir.AluOpType.bypass,
                    replica_groups=[list(range(NCORES))],
                    ins=[h1slab.opt()], outs=[h1tab.opt()])

            # ---------------- P2: layer-1 aggregation + fused layer-2 matmul
            with (
                tc.tile_pool(name="p2g", bufs=6) as gpool,
                tc.tile_pool(name="p2s", bufs=3) as spool,
                tc.tile_pool(name="p2ps", bufs=4, space="PSUM") as pspool,
                tc.tile_pool(name="p2ps2", bufs=4, space="PSUM") as pspool2,
            ):
                for b in range(NB_RUN if "2" in PHASES else 0):
                    wl, wh = int(Wlo[b]), int(Whi[b])
                    wt = wl + wh
                    o = int(offs[b])
                    G = gpool.tile([BLK, wt * ROW], f16, tag="G")
                    G3 = G[:].rearrange("p (w f) -> p w f", f=ROW)
                    for (wbase, wlen, tab) in [(0, wl, h1tab[0:HALF, :]),
                                               (wl, wh, h1tab[HALF:NID, :])]:
                        for w0 in range(0, wlen, GAT_GC):
                            wn = min(GAT_GC, wlen - w0)
                            nc.gpsimd.dma_gather(
                                G3[:, wbase + w0:wbase + w0 + wn, :], tab,
                                idxw_sb[:, (o + wbase + w0) * 8:(o + wbase + w0 + wn) * 8],
                                wn * BLK, wn * BLK, ROW)
                    # logits: leaky(alpha_src + alpha_dst), head-major
                    lst = spool.tile([BLK, wt * HEADS], f32, tag="lst")
                    lsthw = lst[:].rearrange("p (h w) -> p h w", h=HEADS)
                    nc.vector.tensor_tensor(
                        out=lsthw,
                        in0=G3[:, :, BLK:BLK + HEADS].rearrange("p w h -> p h w"),
                        in1=ldc1[:, HEADS * b:HEADS * (b + 1)].unsqueeze(2)
                            .to_broadcast([BLK, HEADS, wt]),
                        op=mybir.AluOpType.add)
                    tmp = spool.tile([BLK, wt * HEADS], f32, tag="tmp")
                    nc.vector.tensor_scalar_mul(out=tmp[:], in0=lst[:],
                                                scalar1=NEG_SLOPE)
                    nc.vector.tensor_tensor(out=lst[:], in0=lst[:], in1=tmp[:],
                                            op=mybir.AluOpType.max)
                    # exact per-dst segment softmax: subtract per-head max
                    nmx = spool.tile([BLK, HEADS], f32, tag="nmx")
                    nc.vector.tensor_reduce(out=nmx[:], in_=lsthw,
                                            axis=mybir.AxisListType.X,
                                            op=mybir.AluOpType.max)
                    nc.vector.tensor_scalar_mul(out=nmx[:], in0=nmx[:],
                                                scalar1=-1.0)
                    # exp, broadcast-expanded to [p, w, h, c] on ACT
                    pexp = spool.tile([BLK, wt * BLK], f16, tag="pexp")
                    pexp4 = pexp[:].rearrange("p (w h c) -> p w h c",
                                              h=HEADS, c=HID)
                    den = spool.tile([BLK, HEADS], f32, tag="den")
                    for h in range(HEADS):
                        nc.scalar.activation(
                            out=pexp4[:, :, h, :],
                            in_=lst[:, h * wt:(h + 1) * wt].unsqueeze(2)
                                .to_broadcast([BLK, wt, HID]),
                            func=mybir.ActivationFunctionType.Exp,
                            bias=nmx[:, h:h + 1],
                            accum_out=den[:, h:h + 1])
                    nc.vector.tensor_scalar_mul(out=den[:], in0=den[:],
                                                scalar1=1.0 / HID)
                    rcp = spool.tile([BLK, HEADS], f32, tag="rcp")
                    nc.vector.reciprocal(out=rcp[:], in_=den[:])
                    # weighted messages in place (both fp16-packed: 2x DVE)
                    nc.vector.tensor_tensor(
                        out=G3[:, :, 0:BLK], in0=G3[:, :, 0:BLK],
                        in1=pexp[:].rearrange("p (w f) -> p w f", f=BLK),
                        op=mybir.AluOpType.mult)
                    w = wt
                    while w > 1:
                        hsz = w // 2
                        nc.vector.tensor_tensor(
                            out=G3[:, 0:hsz, 0:BLK], in0=G3[:, 0:hsz, 0:BLK],
                            in1=G3[:, w - hsz:w, 0:BLK],
                            op=mybir.AluOpType.add)
                        w -= hsz
                    # normalize + bias
                    x2 = spool.tile([BLK, BLK], f16, tag="x2")
                    nc.vector.tensor_tensor(
                        out=x2[:].rearrange("p (h c) -> p h c", c=HID),
                        in0=G3[:, 0, 0:BLK].rearrange("p (h c) -> p h c", c=HID),
                        in1=rcp[:].unsqueeze(2).to_broadcast([BLK, HEADS, HID]),
                        op=mybir.AluOpType.mult)
                    nc.vector.tensor_tensor(out=x2[:], in0=x2[:], in1=b1psb[:],
                                            op=mybir.AluOpType.add)
                    # elu + 1 (the -1 is folded into the layer-2 correction)
                    ex = spool.tile([BLK, BLK], f16, tag="ex")
                    nc.vector.tensor_scalar_min(out=ex[:], in0=x2[:], scalar1=0.0)
                    exf = spool.tile([BLK, BLK], f16, tag="exf")
                    nc.scalar.activation(out=exf[:], in_=ex[:],
                                         func=mybir.ActivationFunctionType.Exp)
                    nc.vector.tensor_scalar_max(out=x2[:], in0=x2[:], scalar1=0.0)
                    nc.vector.tensor_tensor(out=x2[:], in0=x2[:], in1=exf[:],
                                            op=mybir.AluOpType.add)
                    # transpose -> resident x2T slab (fp16)
                    tps = pspool.tile([BLK, BLK], f16, tag="tps")
                    nc.tensor.transpose(out=tps[:], in_=x2[:], identity=ident[:])
                    nc.scalar.activation(out=x2T[:, b * BLK:(b + 1) * BLK],
                                         in_=tps[:],
                                         func=mybir.ActivationFunctionType.Copy)
                    # fused layer-2 slab matmul (+ elu-shift correction)
                    ps2 = pspool2.tile([BLK, 168], f32)
                    nc.tensor.matmul(out=ps2[:],
                                     lhsT=x2T[:, b * BLK:(b + 1) * BLK],
                                     rhs=W2sb[:], start=True, stop=True)
                    h2c = spool.tile([BLK, 168], f16, tag="h2c")
                    nc.vector.tensor_tensor(out=h2c[:], in0=ps2[:], in1=c2sb[:],
                                            op=mybir.AluOpType.add)
                    nc.sync.dma_start(out=h2slab[b * BLK:(b + 1) * BLK, 0:168],
                                      in_=h2c[:])
                    nc.vector.tensor_tensor(out=ld2[:, HEADS * b:HEADS * (b + 1)],
                                            in0=ps2[:, 164:168],
                                            in1=c2sb[:, 164:168],
                                            op=mybir.AluOpType.add)

            if "2" in PHASES:
                nc.sync.dma_start(out=h2slab[SLAB - 1:SLAB, F2:F2 + HEADS],
                                  in_=mrow[:][0:1, :])
            if "3" in PHASES:
                nc.gpsimd.collective_compute(
                    "AllGather", mybir.AluOpType.bypass,
                    replica_groups=[list(range(NCORES))],
                    ins=[h2slab.opt()], outs=[h2tab.opt()])

            # ---------------- P4: layer-2 aggregation -> out
            with (
                tc.tile_pool(name="p4g", bufs=6) as gpool,
                tc.tile_pool(name="p4s", bufs=3) as spool,
            ):
                for b in range(NB_RUN if "4" in PHASES else 0):
                    wl, wh = int(Wlo[b]), int(Whi[b])
                    wt = wl + wh
                    o = int(offs[b])
                    G = gpool.tile([BLK, wt * ROW], f16, tag="G2")
                    G3 = G[:].rearrange("p (w f) -> p w f", f=ROW)
                    for (wbase, wlen, tab) in [(0, wl, h2tab[0:HALF, :]),
                                               (wl, wh, h2tab[HALF:NID, :])]:
                        for w0 in range(0, wlen, GAT_GC):
                            wn = min(GAT_GC, wlen - w0)
                            nc.gpsimd.dma_gather(
                                G3[:, wbase + w0:wbase + w0 + wn, :], tab,
                                idxw_sb[:, (o + wbase + w0) * 8:(o + wbase + w0 + wn) * 8],
                                wn * BLK, wn * BLK, ROW)
                    lst = spool.tile([BLK, wt * HEADS], f32, tag="lst2")
                    lsthw = lst[:].rearrange("p (h w) -> p h w", h=HEADS)
                    nc.vector.tensor_tensor(
                        out=lsthw,
                        in0=G3[:, :, F2:F2 + HEADS].rearrange("p w h -> p h w"),
                        in1=ld2[:, HEADS * b:HEADS * (b + 1)].unsqueeze(2)
                            .to_broadcast([BLK, HEADS, wt]),
                        op=mybir.AluOpType.add)
                    tmp = spool.tile([BLK, wt * HEADS], f32, tag="tmp2")
                    nc.vector.tensor_scalar_mul(out=tmp[:], in0=lst[:],
                                                scalar1=NEG_SLOPE)
                    nc.vector.tensor_tensor(out=lst[:], in0=lst[:], in1=tmp[:],
                                            op=mybir.AluOpType.max)
                    nmx = spool.tile([BLK, HEADS], f32, tag="nmx2")
                    nc.vector.tensor_reduce(out=nmx[:], in_=lsthw,
                                            axis=mybir.AxisListType.X,
                                            op=mybir.AluOpType.max)
                    nc.vector.tensor_scalar_mul(out=nmx[:], in0=nmx[:],
                                                scalar1=-1.0)
                    pexp = spool.tile([BLK, wt * F2], f16, tag="pexp2")
                    pexp4 = pexp[:].rearrange("p (w h c) -> p w h c",
                                              h=HEADS, c=N_CLS)
                    den = spool.tile([BLK, HEADS], f32, tag="den2")
                    for h in range(HEADS):
                        nc.scalar.activation(
                            out=pexp4[:, :, h, :],
                            in_=lst[:, h * wt:(h + 1) * wt].unsqueeze(2)
                                .to_broadcast([BLK, wt, N_CLS]),
                            func=mybir.ActivationFunctionType.Exp,
                            bias=nmx[:, h:h + 1],
                            accum_out=den[:, h:h + 1])
                    nc.vector.tensor_scalar_mul(out=den[:], in0=den[:],
                                                scalar1=1.0 / N_CLS)
                    rcp = spool.tile([BLK, HEADS], f32, tag="rcp2")
                    nc.vector.reciprocal(out=rcp[:], in_=den[:])
                    nc.vector.tensor_tensor(
                        out=G3[:, :, 0:F2], in0=G3[:, :, 0:F2],
                        in1=pexp[:].rearrange("p (w f) -> p w f", f=F2),
                        op=mybir.AluOpType.mult)
                    w = wt
                    while w > 1:
                        hsz = w // 2
                        nc.vector.tensor_tensor(
                            out=G3[:, 0:hsz, 0:F2], in0=G3[:, 0:hsz, 0:F2],
                            in1=G3[:, w - hsz:w, 0:F2],
                            op=mybir.AluOpType.add)
                        w -= hsz
                    ot = spool.tile([BLK, F2], f16, tag="ot")
                    nc.vector.tensor_tensor(
                        out=ot[:].rearrange("p (h c) -> p h c", c=N_CLS),
                        in0=G3[:, 0, 0:F2].rearrange("p (h c) -> p h c", c=N_CLS),
                        in1=rcp[:].unsqueeze(2).to_broadcast([BLK, HEADS, N_CLS]),
                        op=mybir.AluOpType.mult)
                    nc.sync.dma_start(out=t_out[b * BLK:(b + 1) * BLK, :],
                                      in_=ot[:])

    nc.compile()
    return nc


_CACHE = {}


def kernel(**inputs) -> np.ndarray:
    x = np.asarray(inputs["x"], np.float32)
    P = _prep_indices(np.asarray(inputs["edge_index"]))
    node_cid = P["node_cid"]

    g = np.asarray(inputs["bn_gamma"], np.float32)
    bta = np.asarray(inputs["bn_beta"], np.float32)
    mu = np.asarray(inputs["bn_mean"], np.float32)
    var = np.asarray(inputs["bn_var"], np.float32)
    W1 = np.asarray(inputs["W1"], np.float32)
    a1s = np.asarray(inputs["a1_src"], np.float32)
    a1d = np.asarray(inputs["a1_dst"], np.float32)
    W2 = np.asarray(inputs["W2"], np.float32)
    a2s = np.asarray(inputs["a2_src"], np.float32)
    a2d = np.asarray(inputs["a2_dst"], np.float32)

    s = g / np.sqrt(var + BN_EPS)
    W1p = (s[:, None] * W1).astype(np.float32)
    b1p = ((bta - mu * s) @ W1).astype(np.float32)
    A1s = np.zeros((HEADS * HID, HEADS), np.float32)
    A1d = np.zeros((HEADS * HID, HEADS), np.float32)
    A2s = np.zeros((HEADS * N_CLS, HEADS), np.float32)
    A2d = np.zeros((HEADS * N_CLS, HEADS), np.float32)
    for h in range(HEADS):
        A1s[h * HID:(h + 1) * HID, h] = a1s[h]
        A1d[h * HID:(h + 1) * HID, h] = a1d[h]
        A2s[h * N_CLS:(h + 1) * N_CLS, h] = a2s[h]
        A2d[h * N_CLS:(h + 1) * N_CLS, h] = a2d[h]
    W1f = np.concatenate([W1p, W1p @ A1s, W1p @ A1d], axis=1)      # [129, 136]
    csd = (b1p @ A1s + b1p @ A1d).astype(np.float32)               # [4]
    W2f = np.concatenate([W2, W2 @ A2s, W2 @ A2d], axis=1)         # [128, 168]

    # x in cid space
    xp = np.zeros((NID, IN_F), np.float32)
    xp[node_cid] = x

    key = (tuple(P["Wlo"]), tuple(P["Whi"]),
           os.environ.get("GAT_PHASES", "1234"), os.environ.get("GAT_NBLK", ""))
    if key not in _CACHE:
        _CACHE[key] = _build_program(P["Wlo"], P["Whi"], P["offs"], P["S"])
    nc = _CACHE[key]

    common = {
        "W1f": np.ascontiguousarray(W1f[:BLK]).astype(np.float16),
        "W1row": np.tile(W1f[BLK:BLK + 1], (BLK, 1)).astype(np.float32),
        "W2f": W2f.astype(np.float16),
        "csdb": np.tile(csd[None, :], (BLK, 1)).astype(np.float32),
        "b1pb": np.tile(b1p[None, :], (BLK, 1)).astype(np.float16),
        "c2b": np.tile(-W2f.sum(axis=0)[None, :], (BLK, 1)).astype(np.float32),
    }

    in_maps = []
    for q in range(NCORES):
        m = dict(common)
        sl = slice(q * SLAB, (q + 1) * SLAB)
        m["xTs"] = np.ascontiguousarray(xp[sl, 0:BLK].T).astype(np.float16)
        m["xls"] = np.ascontiguousarray(xp[sl, BLK:BLK + 1]).astype(np.float32)
        m["idxc"] = np.ascontiguousarray(P["idxc"][q])
        in_maps.append(m)

    t0 = time.time()
    res = run_bass_kernel_spmd(nc, in_maps, core_ids=list(range(NCORES)))
    global last_run_seconds
    last_run_seconds = time.time() - t0
    outfull = np.concatenate([r["out2"] for r in res.results], axis=0)
    return outfull[node_cid].astype(np.float32)


last_run_seconds = None
